# revision 48
# baseline (speedup 1.0000x reference)
"""Trainium2 Bass kernel for per-head 3-layer MLP + softmax (nn_Clip).

Reference computation (per head h of 16, batch B=32768):
    h1 = relu(emb @ W1[h] + b1[h])          [B, 128]
    h2 = relu(h1 @ W2[h] + b2[h])           [B, 64]
    out[h] = softmax(h2 @ W3[h] + b3[h])    [B, C=10]

Strategy: data-parallel over batch across 8 NeuronCores (4096 rows each),
per-head MLP weights replicated. All matmuls run fp8 with fp32 PSUM
accumulation (host-verified max rel err ~4e-3 vs fp32 reference).

Per 512-row batch tile (feature-major dataflow):
  - Layer 1: psum[d1=128, b=512] per head over 3 fp8 DoubleRowSwInterleave
    chunks (contraction 768 = 3x256); emb shipped pre-transposed fp8.
  - Layer 2: heads paired; head 2j -> psum partitions 0:64, head 2j+1 ->
    64:128 via block-diagonal SwInterleave weights; one matmul per pair.
  - Layer 3: batch-major bf16 matmuls [128, 160] per (pair, m-subtile);
    160-wide moving streams hide the 128-row weight loads (a DoubleRow
    variant with 40-wide streams measured ~35us SLOWER: LDW-serialized).
  - b3 enters multiplicatively after exp (softmax(l+b3) =
    exp(l)*exp(b3)/sum): exp(b3) is materialized once and applied in the
    GpSimd normalize pass, so layer 3 needs no per-tile bias matmuls.
  - relu/bias PSUM->SBUF moves co-bottleneck with the PE: split 14/10
    across ScalarE/VectorE per HW load probes (GpSimd cannot read PSUM).
  - Softmax along the free axis: one merged exp on ScalarE; group sums
    as GpSimd 5+5 add then VectorE reduce; normalize mul on GpSimd;
    per-tile output stores on the GpSimd SWDGE queue (keeping them off
    the SP queue avoids FIFO-blocking the emb prefetch loads).
"""

import numpy as np
import ml_dtypes
from contextlib import ExitStack

from concourse import bacc, bass, mybir, tile
from concourse.bass_utils import run_bass_kernel_spmd

N_CORES = 8
B = 32768
H = 16
E = 768
D1 = 128
D2 = 64
C = 10
B_LOC = B // N_CORES      # 4096 rows per core
B_TILE = 512              # batch tile (matmul free dim)
N_BT = B_LOC // B_TILE    # 8 tiles per core
KC = E // 128             # 6 contraction chunks for layer 1
NPAIR = H // 2            # 8 head pairs
NQ = H // 4               # 4 head quads (pair-pairs) for layer 3
OUTC = H * C              # 160 output columns per row
QC = 4 * C                # 40 logit columns per quad
N_SUB = B_TILE // 128     # 4 batch sub-tiles of 128 for layer 3

BF16 = mybir.dt.bfloat16
F8 = mybir.dt.float8e4
F32 = mybir.dt.float32
AF = mybir.ActivationFunctionType
ALU = mybir.AluOpType
DRSW = mybir.MatmulPerfMode.DoubleRowSwInterleave
DR = mybir.MatmulPerfMode.DoubleRow

_bf = ml_dtypes.bfloat16
_f8 = ml_dtypes.float8_e4m3

# Per-tile engine schedule for the 24 relu+bias PSUM->SBUF moves
# (16 L1 + 8 L2). GPSIMD cannot read PSUM, so only Act/DVE qualify;
# HW probes show Act has more headroom: 14 A / 10 D.
RELU_SCHED = "ADADAADADADAADADADADAADA"
assert len(RELU_SCHED) == 24 and RELU_SCHED.count("A") == 14


def build_program(reps=1):
    nc = bacc.Bacc("TRN2", target_bir_lowering=False, debug=False,
                   num_devices=N_CORES)
    embT = nc.dram_tensor("embT", [E, B_LOC], F8, kind="ExternalInput").ap()
    w1p = nc.dram_tensor("w1p", [128, H * KC * 128], F8, kind="ExternalInput").ap()
    w2p = nc.dram_tensor("w2p", [128, NPAIR * 256], F8, kind="ExternalInput").ap()
    w3p = nc.dram_tensor("w3p", [128, NPAIR * OUTC], BF16, kind="ExternalInput").ap()
    b1p = nc.dram_tensor("b1p", [128, H], F32, kind="ExternalInput").ap()
    b2p = nc.dram_tensor("b2p", [128, NPAIR], F32, kind="ExternalInput").ap()
    b3p = nc.dram_tensor("b3p", [1, 2 * OUTC], BF16, kind="ExternalInput").ap()
    out = nc.dram_tensor("out", [B_LOC, OUTC], F32, kind="ExternalOutput").ap()

    with tile.TileContext(nc) as tc:
        for _ in range(reps):
            with ExitStack() as ctx:
                _body(ctx, tc, embT, w1p, w2p, w3p, b1p, b2p, b3p, out)
    nc.compile()
    return nc


def _body(ctx, tc, embT, w1p, w2p, w3p, b1p, b2p, b3p, out):
    nc = tc.nc
    const = ctx.enter_context(tc.tile_pool(name="const", bufs=2))
    embp = ctx.enter_context(tc.tile_pool(name="embp", bufs=4))
    h1pool = ctx.enter_context(tc.tile_pool(name="h1pool", bufs=8))
    h2pool = ctx.enter_context(tc.tile_pool(name="h2pool", bufs=8))
    smp = ctx.enter_context(tc.tile_pool(name="smp", bufs=2))
    ps1 = ctx.enter_context(tc.tile_pool(name="ps1", bufs=4, space="PSUM"))
    ps2 = ctx.enter_context(tc.tile_pool(name="ps2", bufs=2, space="PSUM"))
    ps3 = ctx.enter_context(tc.tile_pool(name="ps3", bufs=1, space="PSUM"))

    embT3 = embT.rearrange("(k e) b -> e k b", e=128)
    # First emb tile loads before the weights on the SP queue so layer 1
    # can start as early as possible; weights follow on the same queue.
    es0 = embp.tile([128, KC, B_TILE], F8, tag="emb")
    nc.sync.dma_start(es0[:], embT3[:, :, 0:B_TILE])
    b1_sb = const.tile([128, H], F32)
    nc.sync.dma_start(b1_sb[:], b1p[:])
    b2_sb = const.tile([128, NPAIR], F32)
    nc.sync.dma_start(b2_sb[:], b2p[:])
    b3_sb = const.tile([1, 2 * OUTC], BF16)
    nc.sync.dma_start(b3_sb[:], b3p[:])
    w1_sb = const.tile([128, H * KC, 128], F8)
    w1p3 = w1p[:].rearrange("p (t m) -> p t m", m=128)
    for j in range(NPAIR):
        t0 = 2 * j * KC
        t1 = 2 * (j + 1) * KC
        nc.sync.dma_start(w1_sb[:, t0:t1, :], w1p3[:, t0:t1, :])
    w2_sb = const.tile([128, NPAIR, 256], F8)
    nc.sync.dma_start(w2_sb[:], w2p[:].rearrange("p (j t) -> p j t", t=256))
    w3_sb = const.tile([128, NPAIR * OUTC], BF16)
    nc.sync.dma_start(w3_sb[:], w3p[:])
    ones_sb = const.tile([1, 128], BF16)
    nc.vector.memset(ones_sb[:], 1.0)

    # b3 enters the softmax multiplicatively: softmax(l + b3) =
    # exp(l)*exp(b3) / sum(exp(l)*exp(b3)). Materialize exp(b3)
    # broadcast over partitions once (ones-matmul + exp), so layer 3
    # needs no per-tile bias matmuls.
    p2b = ps2.tile([128, B_TILE], F32, tag="p2")
    nc.tensor.matmul(p2b[:, 0:OUTC], ones_sb[:1, :], b3_sb[:1, 0:OUTC],
                     start=True, stop=True)
    eb3_sb = const.tile([128, OUTC], F32)
    nc.scalar.activation(eb3_sb[:], p2b[:, 0:OUTC], AF.Exp)

    for bt in range(N_BT):
        bsl = slice(bt * B_TILE, (bt + 1) * B_TILE)
        if bt == 0:
            es = es0
        else:
            es = embp.tile([128, KC, B_TILE], F8, tag="emb")
            nc.sync.dma_start(es[:], embT3[:, :, bsl])

        p3 = ps3.tile([128, 1024], F32, tag="p3")
        p3off = [0, OUTC, 512, 512 + OUTC]

        nrelu = 0

        def relu(out_ap, in_ap, bias_ap):
            nonlocal nrelu
            eng = RELU_SCHED[nrelu % 24]
            nrelu += 1
            if eng == "A":
                nc.scalar.activation(out_ap, in_ap, AF.Relu, bias=bias_ap)
            else:
                nc.vector.tensor_scalar(out_ap, in_ap, bias_ap,
                                        0.0, ALU.add, ALU.max)

        for j in range(NPAIR):
            h1pair = h1pool.tile([128, 2, B_TILE], F8, tag="h1")
            for hi, h in enumerate((2 * j, 2 * j + 1)):
                p1 = ps1.tile([128, B_TILE], F32, tag="p1")
                for k in range(0, KC, 2):
                    nc.tensor.matmul(
                        p1[:],
                        w1_sb[:, h * KC + k:h * KC + k + 2, :],
                        es[:, k:k + 2, :],
                        start=(k == 0),
                        stop=(k == KC - 2),
                        perf_mode=DRSW,
                    )
                relu(h1pair[:, hi, :], p1[:], b1_sb[:, h:h + 1])

            p2 = ps2.tile([128, B_TILE], F32, tag="p2")
            nc.tensor.matmul(p2[:], w2_sb[:, j, :].rearrange(
                                 "p (t m) -> p t m", m=128),
                             h1pair[:],
                             start=True, stop=True,
                             perf_mode=DRSW)
            h2 = h2pool.tile([128, B_TILE], BF16, tag="h2")
            relu(h2[:], p2[:], b2_sb[:, j:j + 1])
            for m in range(N_SUB):
                nc.tensor.matmul(p3[:, p3off[m]:p3off[m] + OUTC],
                                 h2[:, m * 128:(m + 1) * 128],
                                 w3_sb[:, j * OUTC:(j + 1) * OUTC],
                                 start=(j == 0 and m % 2 == 0),
                                 stop=(j == NPAIR - 1 and m % 2 == 1))

        ex = smp.tile([128, N_SUB * OUTC], F32, tag="ex")
        nc.scalar.activation(
            ex[:].rearrange("p (t c) -> p t c", t=2),
            p3[:].rearrange("p (t c) -> p t c", t=2)[:, :, 0:2 * OUTC],
            AF.Exp)

        # Apply exp(b3) on GpSimd (broadcast over the 4 m-blocks).
        exb = smp.tile([128, N_SUB * OUTC], F32, tag="exb")
        nc.gpsimd.tensor_mul(
            exb[:].rearrange("p (m c) -> p m c", c=OUTC),
            ex[:].rearrange("p (m c) -> p m c", c=OUTC),
            eb3_sb[:][:, None, :].broadcast_to((128, N_SUB, OUTC)),
        )
        ex = exb

        G = N_SUB * H  # 64 softmax groups of width C per partition
        # Group sums over C=10: one GpSimd add halves the width (5+5),
        # then a DVE reduce over the remaining 5.
        exg = ex[:].rearrange("p (g c) -> p g c", c=C)
        s1 = smp.tile([128, G, 5], F32, tag="s1")
        nc.gpsimd.tensor_add(s1[:], exg[:, :, 0:5], exg[:, :, 5:10])
        sums = smp.tile([128, G], F32, tag="sums")
        nc.vector.reduce_sum(sums[:], s1[:], axis=mybir.AxisListType.X)
        nc.vector.reciprocal(sums[:], sums[:])
        outt = smp.tile([128, N_SUB * OUTC], F32, tag="outt")
        nc.gpsimd.tensor_mul(
            outt[:].rearrange("p (g c) -> p g c", c=C),
            ex[:].rearrange("p (g c) -> p g c", c=C),
            sums[:][:, :, None].broadcast_to((128, G, C)),
        )
        nc.gpsimd.dma_start(
            out[bsl, :].rearrange("(m p) c -> p m c", p=128),
            outt[:].rearrange("p (m c) -> p m c", c=OUTC),
        )


def prep_inputs(clip_embedding, W1, b1, W2, b2, W3, b3):
    """Host-side prepack: cast/transpose into the layouts the kernel DMAs."""
    emb = np.asarray(clip_embedding, dtype=np.float32)
    W1 = np.asarray(W1, dtype=np.float32)
    b1 = np.asarray(b1, dtype=np.float32)
    W2 = np.asarray(W2, dtype=np.float32)
    b2 = np.asarray(b2, dtype=np.float32)
    W3 = np.asarray(W3, dtype=np.float32)
    b3 = np.asarray(b3, dtype=np.float32)

    embT = np.ascontiguousarray(emb.astype(_f8).T)              # [768, B]
    # SwInterleave layout per chunk pair (A=chunk k, B=chunk k+1), stored
    # column order [A127, B127, A126, B126, ..., A0, B0] (see bass_interp).
    w1c = W1.astype(np.float32).reshape(H, KC, 128, D1)          # [h,k,e,d]
    w1p = np.zeros((128, H * KC * D1), dtype=np.float32)
    for h in range(H):
        for kp in range(KC // 2):
            A = w1c[h, 2 * kp]       # [e,d] weights for even chunk
            Bm = w1c[h, 2 * kp + 1]  # [e,d] weights for odd chunk
            blk = np.empty((128, 2 * D1), dtype=np.float32)
            blk[:, 0::2] = A[:, ::-1]
            blk[:, 1::2] = Bm[:, ::-1]
            c0 = (h * KC + 2 * kp) * D1
            w1p[:, c0:c0 + 2 * D1] = blk
    w1p = np.ascontiguousarray(w1p.astype(_f8))
    # Block-diagonal per-pair [256, 128] -> SwInterleave storage [128, 256]:
    # stored col 2t = sub0 col (127-t), col 2t+1 = sub1 col (127-t), where
    # sub0 = [W2[2j] | 0] over d1 of head 2j, sub1 = [0 | W2[2j+1]].
    w2p = np.zeros((128, NPAIR * 256), dtype=np.float32)
    for j in range(NPAIR):
        sub0 = np.zeros((128, 128), dtype=np.float32)
        sub1 = np.zeros((128, 128), dtype=np.float32)
        sub0[:, 0:64] = W2[2 * j]
        sub1[:, 64:128] = W2[2 * j + 1]
        blk = np.empty((128, 256), dtype=np.float32)
        blk[:, 0::2] = sub0[:, ::-1]
        blk[:, 1::2] = sub1[:, ::-1]
        w2p[:, j * 256:(j + 1) * 256] = blk
    w2p = np.ascontiguousarray(w2p.astype(_f8))
    w3p = np.zeros((128, NPAIR * OUTC), dtype=_bf)
    for j in range(NPAIR):
        base = j * OUTC
        w3p[0:64, base + 20 * j: base + 20 * j + C] = W3[2 * j].astype(_bf)
        w3p[64:128, base + 20 * j + C: base + 20 * j + 2 * C] = \
            W3[2 * j + 1].astype(_bf)
    b1p = np.ascontiguousarray(b1.T)                            # [128, 16]
    b2p = np.ascontiguousarray(b2.reshape(NPAIR, 128).T)        # [128, 8]
    b3flat = b3.reshape(1, OUTC)
    b3p = np.ascontiguousarray(
        np.concatenate([b3flat, b3flat], axis=1).astype(_bf))   # [1, 320]

    shared = dict(w1p=w1p, w2p=w2p, w3p=w3p, b1p=b1p, b2p=b2p, b3p=b3p)
    in_maps = []
    for c in range(N_CORES):
        m = dict(shared)
        m["embT"] = np.ascontiguousarray(
            embT[:, c * B_LOC:(c + 1) * B_LOC])
        in_maps.append(m)
    return in_maps


def run(inputs, trace=False):
    """Build, compile and run the SPMD kernel; returns (output, results)."""
    in_maps = prep_inputs(
        inputs["clip_embedding"], inputs["W1"], inputs["b1"],
        inputs["W2"], inputs["b2"], inputs["W3"], inputs["b3"])
    nc = build_program()
    res = run_bass_kernel_spmd(nc, in_maps, list(range(N_CORES)), trace=trace)
    outs = [np.asarray(r["out"], dtype=np.float32) for r in res.results]
    full = np.concatenate(outs, axis=0).reshape(B, H, C)
    return full, res


def kernel(**inputs):
    full, _ = run(inputs)
    return full



# revision 50
# speedup vs baseline: 1.3071x; 1.3071x over previous
"""Trainium2 Bass kernel for per-head 3-layer MLP + softmax (nn_Clip).

Reference computation (per head h of 16, batch B=32768):
    h1 = relu(emb @ W1[h] + b1[h])          [B, 128]
    h2 = relu(h1 @ W2[h] + b2[h])           [B, 64]
    out[h] = softmax(h2 @ W3[h] + b3[h])    [B, C=10]

Strategy: data-parallel over batch across 8 NeuronCores (4096 rows each),
per-head MLP weights replicated. All matmuls run fp8 with fp32 PSUM
accumulation (host-verified max rel err ~4e-3 vs fp32 reference).

Per 512-row batch tile (feature-major dataflow):
  - Layer 1: psum[d1=128, b=512] per head over 3 fp8 DoubleRowSwInterleave
    chunks (contraction 768 = 3x256); emb shipped pre-transposed fp8.
  - Layer 2: heads paired; head 2j -> psum partitions 0:64, head 2j+1 ->
    64:128 via block-diagonal SwInterleave weights; one matmul per pair.
  - Layer 3: batch-major bf16 matmuls [128, 160] per (pair, m-subtile);
    160-wide moving streams hide the 128-row weight loads (a DoubleRow
    variant with 40-wide streams measured ~35us SLOWER: LDW-serialized).
  - b3 enters multiplicatively after exp (softmax(l+b3) =
    exp(l)*exp(b3)/sum): exp(b3) is materialized once and applied in the
    GpSimd normalize pass, so layer 3 needs no per-tile bias matmuls.
  - relu/bias PSUM->SBUF moves co-bottleneck with the PE: split 14/10
    across ScalarE/VectorE per HW load probes (GpSimd cannot read PSUM).
  - Softmax along the free axis: one merged exp on ScalarE; group sums
    as GpSimd 5+5 add then VectorE reduce; normalize mul on GpSimd;
    per-tile output stores on the GpSimd SWDGE queue (keeping them off
    the SP queue avoids FIFO-blocking the emb prefetch loads).
"""

import numpy as np
import ml_dtypes
from contextlib import ExitStack

from concourse import bacc, bass, mybir, tile
from concourse.bass_utils import run_bass_kernel_spmd

N_CORES = 8
B = 32768
H = 16
E = 768
D1 = 128
D2 = 64
C = 10
B_LOC = B // N_CORES      # 4096 rows per core
B_TILE = 512              # batch tile (matmul free dim)
N_BT = B_LOC // B_TILE    # 8 tiles per core
KC = E // 128             # 6 contraction chunks for layer 1
NPAIR = H // 2            # 8 head pairs
NQ = H // 4               # 4 head quads (pair-pairs) for layer 3
OUTC = H * C              # 160 output columns per row
QC = 4 * C                # 40 logit columns per quad
N_SUB = B_TILE // 128     # 4 batch sub-tiles of 128 for layer 3

BF16 = mybir.dt.bfloat16
F8 = mybir.dt.float8e4
F32 = mybir.dt.float32
AF = mybir.ActivationFunctionType
ALU = mybir.AluOpType
DRSW = mybir.MatmulPerfMode.DoubleRowSwInterleave
DR = mybir.MatmulPerfMode.DoubleRow

_bf = ml_dtypes.bfloat16
_f8 = ml_dtypes.float8_e4m3

# Per-tile engine schedule for the 24 relu+bias PSUM->SBUF moves
# (16 L1 + 8 L2). GPSIMD cannot read PSUM, so only Act/DVE qualify;
# HW probes show Act has more headroom: 14 A / 10 D.
RELU_SCHED = "ADADAADADADAADADADADAADA"
assert len(RELU_SCHED) == 24 and RELU_SCHED.count("A") == 14


def build_program(reps=1):
    nc = bacc.Bacc("TRN2", target_bir_lowering=False, debug=False,
                   num_devices=N_CORES)
    embT = nc.dram_tensor("embT", [E, B_LOC], F8, kind="ExternalInput").ap()
    w1p = nc.dram_tensor("w1p", [128, H * KC * 128], F8, kind="ExternalInput").ap()
    w2p = nc.dram_tensor("w2p", [128, NPAIR * 256], F8, kind="ExternalInput").ap()
    w3p = nc.dram_tensor("w3p", [128, NPAIR * OUTC], BF16, kind="ExternalInput").ap()
    b1p = nc.dram_tensor("b1p", [128, H], F32, kind="ExternalInput").ap()
    b2p = nc.dram_tensor("b2p", [128, NPAIR], F32, kind="ExternalInput").ap()
    b3p = nc.dram_tensor("b3p", [1, 2 * OUTC], BF16, kind="ExternalInput").ap()
    out = nc.dram_tensor("out", [B_LOC, OUTC], F32, kind="ExternalOutput").ap()

    with tile.TileContext(nc) as tc:
        for _ in range(reps):
            with ExitStack() as ctx:
                _body(ctx, tc, embT, w1p, w2p, w3p, b1p, b2p, b3p, out)
    nc.compile()
    return nc


def _body(ctx, tc, embT, w1p, w2p, w3p, b1p, b2p, b3p, out):
    nc = tc.nc
    const = ctx.enter_context(tc.tile_pool(name="const", bufs=2))
    embp = ctx.enter_context(tc.tile_pool(name="embp", bufs=4))
    h1pool = ctx.enter_context(tc.tile_pool(name="h1pool", bufs=8))
    h2pool = ctx.enter_context(tc.tile_pool(name="h2pool", bufs=8))
    smp = ctx.enter_context(tc.tile_pool(name="smp", bufs=2))
    ps1 = ctx.enter_context(tc.tile_pool(name="ps1", bufs=4, space="PSUM"))
    ps2 = ctx.enter_context(tc.tile_pool(name="ps2", bufs=2, space="PSUM"))
    ps3 = ctx.enter_context(tc.tile_pool(name="ps3", bufs=1, space="PSUM"))

    embT3 = embT.rearrange("(k e) b -> e k b", e=128)
    # First emb tile loads before the weights on the SP queue so layer 1
    # can start as early as possible; weights follow on the same queue.
    es0 = embp.tile([128, KC, B_TILE], F8, tag="emb")
    nc.sync.dma_start(es0[:], embT3[:, :, 0:B_TILE])
    b1_sb = const.tile([128, H], F32)
    nc.sync.dma_start(b1_sb[:], b1p[:])
    b2_sb = const.tile([128, NPAIR], F32)
    nc.sync.dma_start(b2_sb[:], b2p[:])
    b3_sb = const.tile([1, 2 * OUTC], BF16)
    nc.sync.dma_start(b3_sb[:], b3p[:])
    w1_sb = const.tile([128, H * KC, 128], F8)
    w1p3 = w1p[:].rearrange("p (t m) -> p t m", m=128)
    for j in range(NPAIR):
        t0 = 2 * j * KC
        t1 = 2 * (j + 1) * KC
        nc.sync.dma_start(w1_sb[:, t0:t1, :], w1p3[:, t0:t1, :])
    w2_sb = const.tile([128, NPAIR, 256], F8)
    nc.sync.dma_start(w2_sb[:], w2p[:].rearrange("p (j t) -> p j t", t=256))
    w3_sb = const.tile([128, NPAIR * OUTC], BF16)
    nc.sync.dma_start(w3_sb[:], w3p[:])
    ones_sb = const.tile([1, 128], BF16)
    nc.vector.memset(ones_sb[:], 1.0)

    # b3 enters the softmax multiplicatively: softmax(l + b3) =
    # exp(l)*exp(b3) / sum(exp(l)*exp(b3)). Materialize exp(b3)
    # broadcast over partitions once (ones-matmul + exp), so layer 3
    # needs no per-tile bias matmuls.
    p2b = ps2.tile([128, B_TILE], F32, tag="p2")
    nc.tensor.matmul(p2b[:, 0:OUTC], ones_sb[:1, :], b3_sb[:1, 0:OUTC],
                     start=True, stop=True)
    eb3_sb = const.tile([128, OUTC], F32)
    nc.scalar.activation(eb3_sb[:], p2b[:, 0:OUTC], AF.Exp)

    for bt in range(N_BT):
        bsl = slice(bt * B_TILE, (bt + 1) * B_TILE)
        if bt == 0:
            es = es0
        else:
            es = embp.tile([128, KC, B_TILE], F8, tag="emb")
            nc.sync.dma_start(es[:], embT3[:, :, bsl])

        p3 = ps3.tile([128, 1024], F32, tag="p3")
        p3off = [0, OUTC, 512, 512 + OUTC]

        nrelu = 0

        def relu(out_ap, in_ap, bias_ap):
            nonlocal nrelu
            eng = RELU_SCHED[nrelu % 24]
            nrelu += 1
            if eng == "A":
                nc.scalar.activation(out_ap, in_ap, AF.Relu, bias=bias_ap)
            else:
                nc.vector.tensor_scalar(out_ap, in_ap, bias_ap,
                                        0.0, ALU.add, ALU.max)

        for j in range(NPAIR):
            h1pair = h1pool.tile([128, 2, B_TILE], F8, tag="h1")
            for hi, h in enumerate((2 * j, 2 * j + 1)):
                p1 = ps1.tile([128, B_TILE], F32, tag="p1")
                for k in range(0, KC, 2):
                    nc.tensor.matmul(
                        p1[:],
                        w1_sb[:, h * KC + k:h * KC + k + 2, :],
                        es[:, k:k + 2, :],
                        start=(k == 0),
                        stop=(k == KC - 2),
                        perf_mode=DRSW,
                    )
                relu(h1pair[:, hi, :], p1[:], b1_sb[:, h:h + 1])

            p2 = ps2.tile([128, B_TILE], F32, tag="p2")
            nc.tensor.matmul(p2[:], w2_sb[:, j, :].rearrange(
                                 "p (t m) -> p t m", m=128),
                             h1pair[:],
                             start=True, stop=True,
                             perf_mode=DRSW)
            h2 = h2pool.tile([128, B_TILE], BF16, tag="h2")
            relu(h2[:], p2[:], b2_sb[:, j:j + 1])
            for m in range(N_SUB):
                nc.tensor.matmul(p3[:, p3off[m]:p3off[m] + OUTC],
                                 h2[:, m * 128:(m + 1) * 128],
                                 w3_sb[:, j * OUTC:(j + 1) * OUTC],
                                 start=(j == 0 and m % 2 == 0),
                                 stop=(j == NPAIR - 1 and m % 2 == 1))

        ex = smp.tile([128, N_SUB * OUTC], F32, tag="ex")
        nc.scalar.activation(
            ex[:].rearrange("p (t c) -> p t c", t=2),
            p3[:].rearrange("p (t c) -> p t c", t=2)[:, :, 0:2 * OUTC],
            AF.Exp)

        # Apply exp(b3) on GpSimd (broadcast over the 4 m-blocks).
        exb = smp.tile([128, N_SUB * OUTC], F32, tag="exb")
        nc.gpsimd.tensor_mul(
            exb[:].rearrange("p (m c) -> p m c", c=OUTC),
            ex[:].rearrange("p (m c) -> p m c", c=OUTC),
            eb3_sb[:][:, None, :].broadcast_to((128, N_SUB, OUTC)),
        )
        ex = exb

        G = N_SUB * H  # 64 softmax groups of width C per partition
        # Group sums over C=10: one GpSimd add halves the width (5+5),
        # then a DVE reduce over the remaining 5.
        exg = ex[:].rearrange("p (g c) -> p g c", c=C)
        s1 = smp.tile([128, G, 5], F32, tag="s1")
        nc.gpsimd.tensor_add(s1[:], exg[:, :, 0:5], exg[:, :, 5:10])
        sums = smp.tile([128, G], F32, tag="sums")
        nc.vector.reduce_sum(sums[:], s1[:], axis=mybir.AxisListType.X)
        nc.vector.reciprocal(sums[:], sums[:])
        outt = smp.tile([128, N_SUB * OUTC], F32, tag="outt")
        nc.gpsimd.tensor_mul(
            outt[:].rearrange("p (g c) -> p g c", c=C),
            ex[:].rearrange("p (g c) -> p g c", c=C),
            sums[:][:, :, None].broadcast_to((128, G, C)),
        )
        nc.gpsimd.dma_start(
            out[bsl, :].rearrange("(m p) c -> p m c", p=128),
            outt[:].rearrange("p (m c) -> p m c", c=OUTC),
        )


def prep_inputs(clip_embedding, W1, b1, W2, b2, W3, b3):
    """Host-side prepack: cast/transpose into the layouts the kernel DMAs."""
    emb = np.asarray(clip_embedding, dtype=np.float32)
    W1 = np.asarray(W1, dtype=np.float32)
    b1 = np.asarray(b1, dtype=np.float32)
    W2 = np.asarray(W2, dtype=np.float32)
    b2 = np.asarray(b2, dtype=np.float32)
    W3 = np.asarray(W3, dtype=np.float32)
    b3 = np.asarray(b3, dtype=np.float32)

    embT = np.ascontiguousarray(emb.astype(_f8).T)              # [768, B]
    # SwInterleave layout per chunk pair (A=chunk k, B=chunk k+1), stored
    # column order [A127, B127, A126, B126, ..., A0, B0] (see bass_interp).
    w1c = W1.astype(np.float32).reshape(H, KC, 128, D1)          # [h,k,e,d]
    w1p = np.zeros((128, H * KC * D1), dtype=np.float32)
    for h in range(H):
        for kp in range(KC // 2):
            A = w1c[h, 2 * kp]       # [e,d] weights for even chunk
            Bm = w1c[h, 2 * kp + 1]  # [e,d] weights for odd chunk
            blk = np.empty((128, 2 * D1), dtype=np.float32)
            blk[:, 0::2] = A[:, ::-1]
            blk[:, 1::2] = Bm[:, ::-1]
            c0 = (h * KC + 2 * kp) * D1
            w1p[:, c0:c0 + 2 * D1] = blk
    w1p = np.ascontiguousarray(w1p.astype(_f8))
    # Block-diagonal per-pair [256, 128] -> SwInterleave storage [128, 256]:
    # stored col 2t = sub0 col (127-t), col 2t+1 = sub1 col (127-t), where
    # sub0 = [W2[2j] | 0] over d1 of head 2j, sub1 = [0 | W2[2j+1]].
    w2p = np.zeros((128, NPAIR * 256), dtype=np.float32)
    for j in range(NPAIR):
        sub0 = np.zeros((128, 128), dtype=np.float32)
        sub1 = np.zeros((128, 128), dtype=np.float32)
        sub0[:, 0:64] = W2[2 * j]
        sub1[:, 64:128] = W2[2 * j + 1]
        blk = np.empty((128, 256), dtype=np.float32)
        blk[:, 0::2] = sub0[:, ::-1]
        blk[:, 1::2] = sub1[:, ::-1]
        w2p[:, j * 256:(j + 1) * 256] = blk
    w2p = np.ascontiguousarray(w2p.astype(_f8))
    w3p = np.zeros((128, NPAIR * OUTC), dtype=_bf)
    for j in range(NPAIR):
        base = j * OUTC
        w3p[0:64, base + 20 * j: base + 20 * j + C] = W3[2 * j].astype(_bf)
        w3p[64:128, base + 20 * j + C: base + 20 * j + 2 * C] = \
            W3[2 * j + 1].astype(_bf)
    b1p = np.ascontiguousarray(b1.T)                            # [128, 16]
    b2p = np.ascontiguousarray(b2.reshape(NPAIR, 128).T)        # [128, 8]
    b3flat = b3.reshape(1, OUTC)
    b3p = np.ascontiguousarray(
        np.concatenate([b3flat, b3flat], axis=1).astype(_bf))   # [1, 320]

    shared = dict(w1p=w1p, w2p=w2p, w3p=w3p, b1p=b1p, b2p=b2p, b3p=b3p)
    in_maps = []
    for c in range(N_CORES):
        m = dict(shared)
        m["embT"] = np.ascontiguousarray(
            embT[:, c * B_LOC:(c + 1) * B_LOC])
        in_maps.append(m)
    return in_maps


def run(inputs, trace=False):
    """Build, compile and run the SPMD kernel; returns (output, results)."""
    in_maps = prep_inputs(
        inputs["clip_embedding"], inputs["W1"], inputs["b1"],
        inputs["W2"], inputs["b2"], inputs["W3"], inputs["b3"])
    nc = build_program()
    res = run_bass_kernel_spmd(nc, in_maps, list(range(N_CORES)), trace=trace)
    outs = [np.asarray(r["out"], dtype=np.float32) for r in res.results]
    full = np.concatenate(outs, axis=0).reshape(B, H, C)
    return full, res


def kernel(**inputs):
    full, _ = run(inputs)
    return full



# revision 52
# speedup vs baseline: 1.3492x; 1.0322x over previous
"""Trainium2 Bass kernel for per-head 3-layer MLP + softmax (nn_Clip).

Reference computation (per head h of 16, batch B=32768):
    h1 = relu(emb @ W1[h] + b1[h])          [B, 128]
    h2 = relu(h1 @ W2[h] + b2[h])           [B, 64]
    out[h] = softmax(h2 @ W3[h] + b3[h])    [B, C=10]

Strategy: data-parallel over batch across 8 NeuronCores (4096 rows each),
per-head MLP weights replicated. All matmuls run fp8 with fp32 PSUM
accumulation (host-verified max rel err ~4e-3 vs fp32 reference).

Per 512-row batch tile (feature-major dataflow):
  - Layer 1: psum[d1=128, b=512] per head over 3 fp8 DoubleRowSwInterleave
    chunks (contraction 768 = 3x256); emb shipped pre-transposed fp8.
  - Layer 2: heads paired; head 2j -> psum partitions 0:64, head 2j+1 ->
    64:128 via block-diagonal SwInterleave weights; one matmul per pair.
  - Layer 3: batch-major bf16 matmuls [128, 160] per (pair, m-subtile);
    160-wide moving streams hide the 128-row weight loads (a DoubleRow
    variant with 40-wide streams measured ~35us SLOWER: LDW-serialized).
  - b3 enters multiplicatively after exp (softmax(l+b3) =
    exp(l)*exp(b3)/sum): exp(b3) is materialized once and applied in the
    GpSimd normalize pass, so layer 3 needs no per-tile bias matmuls.
  - relu/bias PSUM->SBUF moves co-bottleneck with the PE: split 14/10
    across ScalarE/VectorE per HW load probes (GpSimd cannot read PSUM).
  - Softmax along the free axis: one merged exp on ScalarE; group sums
    as GpSimd 5+5 add then VectorE reduce; normalize mul on GpSimd;
    per-tile output stores on the GpSimd SWDGE queue (keeping them off
    the SP queue avoids FIFO-blocking the emb prefetch loads).
"""

import numpy as np
import ml_dtypes
from contextlib import ExitStack

from concourse import bacc, bass, mybir, tile
from concourse.bass_utils import run_bass_kernel_spmd

N_CORES = 8
B = 32768
H = 16
E = 768
D1 = 128
D2 = 64
C = 10
B_LOC = B // N_CORES      # 4096 rows per core
B_TILE = 512              # batch tile (matmul free dim)
N_BT = B_LOC // B_TILE    # 8 tiles per core
KC = E // 128             # 6 contraction chunks for layer 1
NPAIR = H // 2            # 8 head pairs
NQ = H // 4               # 4 head quads (pair-pairs) for layer 3
OUTC = H * C              # 160 output columns per row
QC = 4 * C                # 40 logit columns per quad
N_SUB = B_TILE // 128     # 4 batch sub-tiles of 128 for layer 3

BF16 = mybir.dt.bfloat16
F8 = mybir.dt.float8e4
F32 = mybir.dt.float32
AF = mybir.ActivationFunctionType
ALU = mybir.AluOpType
DRSW = mybir.MatmulPerfMode.DoubleRowSwInterleave
DR = mybir.MatmulPerfMode.DoubleRow

_bf = ml_dtypes.bfloat16
_f8 = ml_dtypes.float8_e4m3

# Per-tile engine schedule for the 24 relu+bias PSUM->SBUF moves
# (16 L1 + 8 L2). GPSIMD cannot read PSUM, so only Act/DVE qualify;
# HW probes show Act has more headroom: 14 A / 10 D.
RELU_SCHED = "ADADAADADADAADADADADAADA"
assert len(RELU_SCHED) == 24 and RELU_SCHED.count("A") == 14


def build_program(reps=1):
    nc = bacc.Bacc("TRN2", target_bir_lowering=False, debug=False,
                   num_devices=N_CORES)
    embT = nc.dram_tensor("embT", [E, B_LOC], F8, kind="ExternalInput").ap()
    w1p = nc.dram_tensor("w1p", [128, H * KC * 128], F8, kind="ExternalInput").ap()
    w2p = nc.dram_tensor("w2p", [128, NPAIR * 256], F8, kind="ExternalInput").ap()
    w3p = nc.dram_tensor("w3p", [128, NPAIR * OUTC], BF16, kind="ExternalInput").ap()
    b1p = nc.dram_tensor("b1p", [128, H], F32, kind="ExternalInput").ap()
    b2p = nc.dram_tensor("b2p", [128, NPAIR], F32, kind="ExternalInput").ap()
    b3p = nc.dram_tensor("b3p", [1, 2 * OUTC], BF16, kind="ExternalInput").ap()
    out = nc.dram_tensor("out", [B_LOC, OUTC], F32, kind="ExternalOutput").ap()

    with tile.TileContext(nc) as tc:
        for _ in range(reps):
            with ExitStack() as ctx:
                _body(ctx, tc, embT, w1p, w2p, w3p, b1p, b2p, b3p, out)
    nc.compile()
    return nc


def _body(ctx, tc, embT, w1p, w2p, w3p, b1p, b2p, b3p, out):
    nc = tc.nc
    const = ctx.enter_context(tc.tile_pool(name="const", bufs=2))
    embp = ctx.enter_context(tc.tile_pool(name="embp", bufs=4))
    h1pool = ctx.enter_context(tc.tile_pool(name="h1pool", bufs=8))
    h2pool = ctx.enter_context(tc.tile_pool(name="h2pool", bufs=8))
    smp = ctx.enter_context(tc.tile_pool(name="smp", bufs=2))
    ps1 = ctx.enter_context(tc.tile_pool(name="ps1", bufs=4, space="PSUM"))
    ps2 = ctx.enter_context(tc.tile_pool(name="ps2", bufs=2, space="PSUM"))
    ps3 = ctx.enter_context(tc.tile_pool(name="ps3", bufs=1, space="PSUM"))

    embT3 = embT.rearrange("(k e) b -> e k b", e=128)
    # First emb tile loads before the weights on the SP queue so layer 1
    # can start as early as possible; weights follow on the same queue.
    es0 = embp.tile([128, KC, B_TILE], F8, tag="emb")
    nc.sync.dma_start(es0[:], embT3[:, :, 0:B_TILE])
    b1_sb = const.tile([128, H], F32)
    nc.sync.dma_start(b1_sb[:], b1p[:])
    b2_sb = const.tile([128, NPAIR], F32)
    nc.sync.dma_start(b2_sb[:], b2p[:])
    b3_sb = const.tile([1, 2 * OUTC], BF16)
    nc.sync.dma_start(b3_sb[:], b3p[:])
    w1_sb = const.tile([128, H * KC, 128], F8)
    w1p3 = w1p[:].rearrange("p (t m) -> p t m", m=128)
    for j in range(NPAIR):
        t0 = 2 * j * KC
        t1 = 2 * (j + 1) * KC
        nc.sync.dma_start(w1_sb[:, t0:t1, :], w1p3[:, t0:t1, :])
    w2_sb = const.tile([128, NPAIR, 256], F8)
    nc.sync.dma_start(w2_sb[:], w2p[:].rearrange("p (j t) -> p j t", t=256))
    w3_sb = const.tile([128, NPAIR * OUTC], BF16)
    nc.sync.dma_start(w3_sb[:], w3p[:])
    ones_sb = const.tile([1, 128], BF16)
    nc.vector.memset(ones_sb[:], 1.0)

    # b3 enters the softmax multiplicatively: softmax(l + b3) =
    # exp(l)*exp(b3) / sum(exp(l)*exp(b3)). Materialize exp(b3)
    # broadcast over partitions once (ones-matmul + exp), so layer 3
    # needs no per-tile bias matmuls.
    p2b = ps2.tile([128, B_TILE], F32, tag="p2")
    nc.tensor.matmul(p2b[:, 0:OUTC], ones_sb[:1, :], b3_sb[:1, 0:OUTC],
                     start=True, stop=True)
    eb3_sb = const.tile([128, OUTC], F32)
    nc.scalar.activation(eb3_sb[:], p2b[:, 0:OUTC], AF.Exp)

    for bt in range(N_BT):
        bsl = slice(bt * B_TILE, (bt + 1) * B_TILE)
        if bt == 0:
            es = es0
        else:
            es = embp.tile([128, KC, B_TILE], F8, tag="emb")
            nc.sync.dma_start(es[:], embT3[:, :, bsl])

        p3 = ps3.tile([128, 1024], F32, tag="p3")
        p3off = [0, OUTC, 512, 512 + OUTC]

        nrelu = 0

        def relu(out_ap, in_ap, bias_ap):
            nonlocal nrelu
            eng = RELU_SCHED[nrelu % 24]
            nrelu += 1
            if eng == "A":
                nc.scalar.activation(out_ap, in_ap, AF.Relu, bias=bias_ap)
            else:
                nc.vector.tensor_scalar(out_ap, in_ap, bias_ap,
                                        0.0, ALU.add, ALU.max)

        for j in range(NPAIR):
            h1pair = h1pool.tile([128, 2, B_TILE], F8, tag="h1")
            for hi, h in enumerate((2 * j, 2 * j + 1)):
                p1 = ps1.tile([128, B_TILE], F32, tag="p1")
                for k in range(0, KC, 2):
                    nc.tensor.matmul(
                        p1[:],
                        w1_sb[:, h * KC + k:h * KC + k + 2, :],
                        es[:, k:k + 2, :],
                        start=(k == 0),
                        stop=(k == KC - 2),
                        perf_mode=DRSW,
                    )
                relu(h1pair[:, hi, :], p1[:], b1_sb[:, h:h + 1])

            p2 = ps2.tile([128, B_TILE], F32, tag="p2")
            nc.tensor.matmul(p2[:], w2_sb[:, j, :].rearrange(
                                 "p (t m) -> p t m", m=128),
                             h1pair[:],
                             start=True, stop=True,
                             perf_mode=DRSW)
            h2 = h2pool.tile([128, B_TILE], BF16, tag="h2")
            relu(h2[:], p2[:], b2_sb[:, j:j + 1])
            for m in range(N_SUB):
                nc.tensor.matmul(p3[:, p3off[m]:p3off[m] + OUTC],
                                 h2[:, m * 128:(m + 1) * 128],
                                 w3_sb[:, j * OUTC:(j + 1) * OUTC],
                                 start=(j == 0 and m % 2 == 0),
                                 stop=(j == NPAIR - 1 and m % 2 == 1))

        ex = smp.tile([128, N_SUB * OUTC], F32, tag="ex")
        nc.scalar.activation(
            ex[:].rearrange("p (t c) -> p t c", t=2),
            p3[:].rearrange("p (t c) -> p t c", t=2)[:, :, 0:2 * OUTC],
            AF.Exp)

        # Apply exp(b3) on GpSimd (broadcast over the 4 m-blocks).
        exb = smp.tile([128, N_SUB * OUTC], F32, tag="exb")
        nc.gpsimd.tensor_mul(
            exb[:].rearrange("p (m c) -> p m c", c=OUTC),
            ex[:].rearrange("p (m c) -> p m c", c=OUTC),
            eb3_sb[:][:, None, :].broadcast_to((128, N_SUB, OUTC)),
        )
        ex = exb

        G = N_SUB * H  # 64 softmax groups of width C per partition
        # Group sums over C=10: one GpSimd add halves the width (5+5),
        # then a DVE reduce over the remaining 5.
        exg = ex[:].rearrange("p (g c) -> p g c", c=C)
        s1 = smp.tile([128, G, 5], F32, tag="s1")
        nc.gpsimd.tensor_add(s1[:], exg[:, :, 0:5], exg[:, :, 5:10])
        sums = smp.tile([128, G], F32, tag="sums")
        nc.vector.reduce_sum(sums[:], s1[:], axis=mybir.AxisListType.X)
        nc.vector.reciprocal(sums[:], sums[:])
        outt = smp.tile([128, N_SUB * OUTC], F32, tag="outt")
        nc.gpsimd.tensor_mul(
            outt[:].rearrange("p (g c) -> p g c", c=C),
            ex[:].rearrange("p (g c) -> p g c", c=C),
            sums[:][:, :, None].broadcast_to((128, G, C)),
        )
        nc.gpsimd.dma_start(
            out[bsl, :].rearrange("(m p) c -> p m c", p=128),
            outt[:].rearrange("p (m c) -> p m c", c=OUTC),
        )


def prep_inputs(clip_embedding, W1, b1, W2, b2, W3, b3):
    """Host-side prepack: cast/transpose into the layouts the kernel DMAs."""
    emb = np.asarray(clip_embedding, dtype=np.float32)
    W1 = np.asarray(W1, dtype=np.float32)
    b1 = np.asarray(b1, dtype=np.float32)
    W2 = np.asarray(W2, dtype=np.float32)
    b2 = np.asarray(b2, dtype=np.float32)
    W3 = np.asarray(W3, dtype=np.float32)
    b3 = np.asarray(b3, dtype=np.float32)

    embT = np.ascontiguousarray(emb.astype(_f8).T)              # [768, B]
    # SwInterleave layout per chunk pair (A=chunk k, B=chunk k+1), stored
    # column order [A127, B127, A126, B126, ..., A0, B0] (see bass_interp).
    w1c = W1.astype(np.float32).reshape(H, KC, 128, D1)          # [h,k,e,d]
    w1p = np.zeros((128, H * KC * D1), dtype=np.float32)
    for h in range(H):
        for kp in range(KC // 2):
            A = w1c[h, 2 * kp]       # [e,d] weights for even chunk
            Bm = w1c[h, 2 * kp + 1]  # [e,d] weights for odd chunk
            blk = np.empty((128, 2 * D1), dtype=np.float32)
            blk[:, 0::2] = A[:, ::-1]
            blk[:, 1::2] = Bm[:, ::-1]
            c0 = (h * KC + 2 * kp) * D1
            w1p[:, c0:c0 + 2 * D1] = blk
    w1p = np.ascontiguousarray(w1p.astype(_f8))
    # Block-diagonal per-pair [256, 128] -> SwInterleave storage [128, 256]:
    # stored col 2t = sub0 col (127-t), col 2t+1 = sub1 col (127-t), where
    # sub0 = [W2[2j] | 0] over d1 of head 2j, sub1 = [0 | W2[2j+1]].
    w2p = np.zeros((128, NPAIR * 256), dtype=np.float32)
    for j in range(NPAIR):
        sub0 = np.zeros((128, 128), dtype=np.float32)
        sub1 = np.zeros((128, 128), dtype=np.float32)
        sub0[:, 0:64] = W2[2 * j]
        sub1[:, 64:128] = W2[2 * j + 1]
        blk = np.empty((128, 256), dtype=np.float32)
        blk[:, 0::2] = sub0[:, ::-1]
        blk[:, 1::2] = sub1[:, ::-1]
        w2p[:, j * 256:(j + 1) * 256] = blk
    w2p = np.ascontiguousarray(w2p.astype(_f8))
    w3p = np.zeros((128, NPAIR * OUTC), dtype=_bf)
    for j in range(NPAIR):
        base = j * OUTC
        w3p[0:64, base + 20 * j: base + 20 * j + C] = W3[2 * j].astype(_bf)
        w3p[64:128, base + 20 * j + C: base + 20 * j + 2 * C] = \
            W3[2 * j + 1].astype(_bf)
    b1p = np.ascontiguousarray(b1.T)                            # [128, 16]
    b2p = np.ascontiguousarray(b2.reshape(NPAIR, 128).T)        # [128, 8]
    b3flat = b3.reshape(1, OUTC)
    b3p = np.ascontiguousarray(
        np.concatenate([b3flat, b3flat], axis=1).astype(_bf))   # [1, 320]

    shared = dict(w1p=w1p, w2p=w2p, w3p=w3p, b1p=b1p, b2p=b2p, b3p=b3p)
    in_maps = []
    for c in range(N_CORES):
        m = dict(shared)
        m["embT"] = np.ascontiguousarray(
            embT[:, c * B_LOC:(c + 1) * B_LOC])
        in_maps.append(m)
    return in_maps


def run(inputs, trace=False):
    """Build, compile and run the SPMD kernel; returns (output, results)."""
    in_maps = prep_inputs(
        inputs["clip_embedding"], inputs["W1"], inputs["b1"],
        inputs["W2"], inputs["b2"], inputs["W3"], inputs["b3"])
    nc = build_program()
    res = run_bass_kernel_spmd(nc, in_maps, list(range(N_CORES)), trace=trace)
    outs = [np.asarray(r["out"], dtype=np.float32) for r in res.results]
    full = np.concatenate(outs, axis=0).reshape(B, H, C)
    return full, res


def kernel(**inputs):
    full, _ = run(inputs)
    return full

